# revision 2
# baseline (speedup 1.0000x reference)
"""Trainium2 Bass kernel for ContextualLoss — v2.

Contract: kernel(**inputs) takes FULL inputs {"inputs": [8,128,64,64] f32,
"targets": [8,128,64,64] f32} and returns the FULL scalar loss (np.float32).

Sharding: data-parallel over batch B=8 across the 8 NeuronCores. Host
computes the cross-batch target channel mean y_mu and averages the 8
per-batch scalar losses.

Per-core math (x, y: [C=128, N=4096], mu: [128,1]):
    xc = x - mu ; yc = y - mu                       (bf16)
    v[m] = 1/max(||yc[:,m]||, 1e-12); u[n] likewise for xc
    ycv = yc * v[m]  (v broadcast to a row via PE-transpose + DMA reshape +
                      Pool partition_broadcast)
    Sv = xc_blk^T @ ycv                             (PE -> PSUM = G*v)
    drain Sv -> SBUF bf16 (split: DVE tensor_copy chunk0 / ACT copy chunk1)
    gv[n] = rowmax Sv  (DVE pairwise tensor_tensor-max tree @2x)
    sc = 1/(h(1+eps) - h*u*gv); se = sc*u; bv = 1/h - sc
    w = exp(se*Sv + bv)  (ACT, fused rowsum accumulator -> r)
    wn = w / r           (DVE tensor_scalar @4x)
    colrow_nb[m] = max_n wn  (Pool partition_all_reduce) -> DMA to
                   collector partition nb
    tail: par over the 32 collector rows -> mean -> -log
"""

import numpy as np

import concourse.bass as bass
import concourse.tile as tile
from concourse import bacc, mybir
from concourse.bass_utils import run_bass_kernel_spmd

F32 = mybir.dt.float32
BF16 = mybir.dt.bfloat16
AF = mybir.ActivationFunctionType
OP = mybir.AluOpType
RED = bass.bass_isa.ReduceOp

B, C, H, W = 8, 128, 64, 64
N = H * W                  # 4096
P = 128
NBLK = N // P              # 32 row blocks
MM_N = 512                 # matmul moving free dim (one PSUM bank)
MC = 2048                  # PSUM chunk (4 banks); 2 chunks per block
DD = 1792                  # DVE drain width within chunk0 (rest + chunk1: ACT)
H_BW = 0.5
EPS = 1e-5
NORM_EPS = 1e-12
NEG_INF = -3.0e38
N_CORES = 8
PRE_CH = 1024              # preamble processing chunk


def _kernel_body(tc):
    nc = tc.nc
    x_d = nc.dram_tensor("x", [P, N], F32, kind="ExternalInput").ap()
    y_d = nc.dram_tensor("y", [P, N], F32, kind="ExternalInput").ap()
    mu_d = nc.dram_tensor("mu", [P, 1], F32, kind="ExternalInput").ap()
    id_d = nc.dram_tensor("ident", [P, P], F32, kind="ExternalInput").ap()
    loss_d = nc.dram_tensor("loss", [1, 1], F32, kind="ExternalOutput").ap()
    vs_d = nc.dram_tensor("vscratch", [1, N], BF16, kind="Internal").ap()

    from contextlib import ExitStack
    with ExitStack() as ctx:
        persist = ctx.enter_context(tc.tile_pool(name="persist", bufs=1))
        small = ctx.enter_context(tc.tile_pool(name="small", bufs=8))

        # ---- constants ----
        mu_sb = persist.tile([P, 1], F32)
        nc.sync.dma_start(mu_sb[:], mu_d)
        negmu = persist.tile([P, 1], F32)
        nc.vector.tensor_scalar_mul(negmu[:], mu_sb[:], -1.0)
        ident_f = persist.tile([P, P], F32)
        nc.sync.dma_start(ident_f[:], id_d)
        ones_col_bf = persist.tile([P, 1], BF16)
        nc.vector.memset(ones_col_bf[:], 1.0)
        c_hbias = persist.tile([P, 1], F32)
        nc.vector.memset(c_hbias[:], H_BW * (1.0 + EPS))
        c_invh = persist.tile([P, 1], F32)
        nc.vector.memset(c_invh[:], 1.0 / H_BW)
        c_eps = persist.tile([P, 1], F32)
        nc.vector.memset(c_eps[:], EPS)

        xc = persist.tile([P, N], BF16)
        ycv = persist.tile([P, N], BF16)
        collector = persist.tile([P, N], BF16)
        u_col = persist.tile([P, NBLK], F32)
        hu_col = persist.tile([P, NBLK], F32)

        # ---------- preamble: load + center + norms ----------
        NCH = N // PRE_CH
        with tc.tile_pool(name="load", bufs=1) as load, \
             tc.tile_pool(name="ssq_ps", bufs=1, space="PSUM") as sp, \
             tc.tile_pool(name="sq_pool", bufs=2) as sqp:
            y_sb = load.tile([P, N], F32)
            x_sb = load.tile([P, N], F32)
            yc = load.tile([P, N], BF16)
            ssq_y = sp.tile([P, NBLK], F32)
            ssq_x = sp.tile([P, NBLK], F32)
            # y chain first (feeds v -> vrow -> ycv, the loop's rhs)
            for c in range(NCH):
                s = c * PRE_CH
                nc.sync.dma_start(y_sb[:, s:s + PRE_CH], y_d[:, s:s + PRE_CH])
            for c in range(NCH):
                s = c * PRE_CH
                nc.sync.dma_start(x_sb[:, s:s + PRE_CH], x_d[:, s:s + PRE_CH])
            for c in range(NCH):
                s = c * PRE_CH
                # center on ACT (bias = -mu), square on DVE (bf16 2x)
                nc.scalar.activation(yc[:, s:s + PRE_CH], y_sb[:, s:s + PRE_CH],
                                     AF.Identity, bias=negmu[:, 0:1], scale=1.0)
                ysq = sqp.tile([P, PRE_CH], BF16, name="ysq")
                nc.vector.tensor_mul(ysq[:], yc[:, s:s + PRE_CH],
                                     yc[:, s:s + PRE_CH])
                for j in range(PRE_CH // P):
                    col = c * (PRE_CH // P) + j
                    nc.tensor.matmul(ssq_y[:, col:col + 1],
                                     ysq[:, j * P:(j + 1) * P],
                                     ones_col_bf[:], start=True, stop=True)
            # v chain: inv-norm of yc columns, in [P, NBLK] layout
            v_col = small.tile([P, NBLK], F32, name="v_col")
            nc.scalar.activation(v_col[:], ssq_y[:], AF.Sqrt)
            nc.vector.tensor_scalar_max(v_col[:], v_col[:], NORM_EPS)
            nc.vector.reciprocal(v_col[:], v_col[:])
            v_bf = small.tile([P, NBLK], BF16, name="v_bf")
            nc.vector.tensor_copy(v_bf[:], v_col[:])

            # x chain (overlaps): center on DVE, square on DVE
            for c in range(NCH):
                s = c * PRE_CH
                nc.vector.tensor_scalar_sub(xc[:, s:s + PRE_CH],
                                            x_sb[:, s:s + PRE_CH],
                                            mu_sb[:, 0:1])
                xsq = sqp.tile([P, PRE_CH], BF16, name="xsq")
                nc.vector.tensor_mul(xsq[:], xc[:, s:s + PRE_CH],
                                     xc[:, s:s + PRE_CH])
                for j in range(PRE_CH // P):
                    col = c * (PRE_CH // P) + j
                    nc.tensor.matmul(ssq_x[:, col:col + 1],
                                     xsq[:, j * P:(j + 1) * P],
                                     ones_col_bf[:], start=True, stop=True)
            nc.scalar.activation(u_col[:], ssq_x[:], AF.Sqrt)
            nc.vector.tensor_scalar_max(u_col[:], u_col[:], NORM_EPS)
            nc.vector.reciprocal(u_col[:], u_col[:])
            nc.vector.tensor_scalar_mul(hu_col[:], u_col[:], -H_BW)

            # ---- vrow: v_bf [P, NBLK] -> row [1, N] -> broadcast [P, N] ----
            with tc.tile_pool(name="vt_ps", bufs=1, space="PSUM") as vtp:
                # PE transpose: vT[j, p] = v[p, j]  ([NBLK, P] f32 in PSUM)
                ident_bf = small.tile([P, P], BF16, name="ident_bf")
                nc.vector.tensor_copy(ident_bf[:], ident_f[:])
                vT_ps = vtp.tile([NBLK, P], F32)
                nc.tensor.transpose(vT_ps[:], v_col[:], ident_f[:])
                vT = small.tile([NBLK, P], BF16, name="vT")
                nc.vector.tensor_copy(vT[:], vT_ps[:])
            vseed = persist.tile([1, N], BF16)
            # reshape [NBLK,P] -> [1,N] via a DRAM bounce (linear layout):
            # write vT rows contiguously, read back as one 4096-elem row
            nc.sync.dma_start(vs_d.rearrange("o (j p) -> (o j) p", p=P),
                              vT[:])
            nc.sync.dma_start(vseed[0:1, :], vs_d)
            vrow = load.tile([P, N], BF16)
            # chunked broadcast + ycv so the first matmuls can start early
            for c in range(NCH):
                s = c * PRE_CH
                nc.gpsimd.partition_broadcast(vrow[:, s:s + PRE_CH],
                                              vseed[0:1, s:s + PRE_CH])
                nc.vector.tensor_mul(ycv[:, s:s + PRE_CH],
                                     yc[:, s:s + PRE_CH], vrow[:, s:s + PRE_CH])

        # ---------- main loop over 32 row blocks ----------
        with tc.tile_pool(name="g_ps_pool", bufs=2, space="PSUM") as gp, \
             tc.tile_pool(name="sv_pool", bufs=4) as svp, \
             tc.tile_pool(name="w_pool", bufs=4) as wp, \
             tc.tile_pool(name="tree_pool", bufs=3) as trp, \
             tc.tile_pool(name="par_pool", bufs=3) as pp:

            def block_exp(st):
                # exp(nb') for a previous block (ACT; overlaps this block's mm)
                nb_, sv_, se_, bv_ = st
                w = wp.tile([P, N], BF16, name="w")
                r = small.tile([P, 1], F32, name="r")
                nc.scalar.activation(w[:], sv_[:], AF.Exp, bias=bv_[:, 0:1],
                                     scale=se_[:, 0:1], accum_out=r[:, 0:1])
                return nb_, w, r

            def block_norm(st):
                # rinv/wn/par/dma for a previous block (DVE late + Pool)
                nb_, w, r = st
                rinv = small.tile([P, 1], F32, name="rinv")
                nc.vector.reciprocal(rinv[:], r[:])
                # wn = w * rinv in place (@4x)
                nc.vector.tensor_scalar_mul(w[:], w[:], rinv[:, 0:1])
                # column max over the block's 128 rows -> one row
                parout = pp.tile([P, N], BF16, name="parout")
                nc.gpsimd.partition_all_reduce(parout[:], w[:], P, RED.max)
                nc.sync.dma_start(collector[nb_:nb_ + 1, :], parout[0:1, :])

            prev = None       # block awaiting exp (1 behind)
            prev2 = None      # block awaiting rinv/wn/par (2 behind)
            for nb in range(NBLK):
                lhsT = xc[:, nb * P:(nb + 1) * P]
                sv = svp.tile([P, N], BF16, name="sv")
                for ch in range(N // MC):
                    g = gp.tile([P, MC], F32, name="g")
                    for j in range(MC // MM_N):
                        m0 = ch * MC + j * MM_N
                        nc.tensor.matmul(g[:, j * MM_N:(j + 1) * MM_N], lhsT,
                                         ycv[:, m0:m0 + MM_N],
                                         start=True, stop=True)
                    if ch == 0:
                        # drain split: DVE takes [0:DD]; ACT the rest
                        nc.vector.tensor_copy(sv[:, 0:DD], g[:, 0:DD])
                        nc.scalar.activation(sv[:, DD:MC], g[:, DD:MC],
                                             AF.Copy)
                    else:
                        nc.scalar.activation(sv[:, MC:N], g[:], AF.Copy)
                # exp(nb-1) AFTER the drains: keeps ACT's drains at higher
                # scheduler priority so the next block's tree starts early
                if prev is not None:
                    prev = block_exp(prev)

                # rowmax via pairwise tt-max tree (bf16 @2x)
                m1 = trp.tile([P, MC], BF16, name="m1")
                nc.vector.tensor_max(m1[:], sv[:, 0:MC], sv[:, MC:N])
                wdt = MC // 2
                while wdt >= 64:
                    nc.vector.tensor_max(m1[:, 0:wdt], m1[:, 0:wdt],
                                         m1[:, wdt:2 * wdt])
                    wdt //= 2
                gv = small.tile([P, 1], F32, name="gv")
                nc.vector.reduce_max(gv[:], m1[:, 0:64],
                                     axis=mybir.AxisListType.X)

                # small chain: sc = 1/(h(1+eps) - h*u*gv); se = sc*u;
                # bv = 1/h - sc
                t = small.tile([P, 1], F32, name="t")
                nc.vector.tensor_scalar(t[:], gv[:], hu_col[:, nb:nb + 1],
                                        c_hbias[:, 0:1], OP.mult, OP.add)
                sc = small.tile([P, 1], F32, name="sc")
                nc.vector.reciprocal(sc[:], t[:])
                se = small.tile([P, 1], F32, name="se")
                nc.vector.tensor_scalar_mul(se[:], sc[:], u_col[:, nb:nb + 1])
                bv = small.tile([P, 1], F32, name="bv")
                nc.vector.tensor_scalar(bv[:], sc[:], -1.0, c_invh[:, 0:1],
                                        OP.mult, OP.add)
                if prev2 is not None:
                    block_norm(prev2)  # DVE late: exp(nb-2) done long ago
                prev2 = prev
                prev = (nb, sv, se, bv)
            block_norm(prev2)          # block NBLK-2 (already exp'd)
            prev = block_exp(prev)     # block NBLK-1
            block_norm(prev)

        # ---------- tail ----------
        cmax = persist.tile([NBLK, N], BF16)
        nc.gpsimd.partition_all_reduce(cmax[:], collector[0:NBLK, :], NBLK,
                                       RED.max)
        cm_sum = persist.tile([1, 1], F32)
        nc.vector.reduce_sum(cm_sum[:], cmax[0:1, :],
                             axis=mybir.AxisListType.X)
        lnv = persist.tile([1, 1], F32)
        nc.scalar.activation(lnv[:], cm_sum[:], AF.Ln,
                             bias=c_eps[0:1, 0:1], scale=1.0 / N)
        loss_sb = persist.tile([1, 1], F32)
        nc.vector.tensor_scalar_mul(loss_sb[:], lnv[:], -1.0)
        nc.sync.dma_start(loss_d, loss_sb[:])


_NC_CACHE = None


def _get_nc():
    global _NC_CACHE
    if _NC_CACHE is None:
        nc = bacc.Bacc("TRN2", target_bir_lowering=False, debug=False)
        with tile.TileContext(nc) as tc:
            _kernel_body(tc)
        nc.compile()
        _NC_CACHE = nc
    return _NC_CACHE


def kernel(inputs, targets):
    x = np.ascontiguousarray(np.asarray(inputs, dtype=np.float32))
    y = np.ascontiguousarray(np.asarray(targets, dtype=np.float32))
    assert x.shape == (B, C, H, W) and y.shape == (B, C, H, W)
    mu = y.mean(axis=(0, 2, 3)).astype(np.float32).reshape(C, 1)
    in_maps = [
        {
            "x": x[b].reshape(C, N),
            "y": y[b].reshape(C, N),
            "mu": mu,
            "ident": np.eye(P, dtype=np.float32),
        }
        for b in range(B)
    ]
    nc = _get_nc()
    res = run_bass_kernel_spmd(nc, in_maps, list(range(N_CORES)))
    losses = [float(res.results[b]["loss"][0, 0]) for b in range(B)]
    return np.float32(np.mean(losses))


# revision 4
# speedup vs baseline: 1.0716x; 1.0716x over previous
"""Trainium2 Bass kernel for ContextualLoss — v2.

Contract: kernel(**inputs) takes FULL inputs {"inputs": [8,128,64,64] f32,
"targets": [8,128,64,64] f32} and returns the FULL scalar loss (np.float32).

Sharding: data-parallel over batch B=8 across the 8 NeuronCores. Host
computes the cross-batch target channel mean y_mu and averages the 8
per-batch scalar losses.

Per-core math (x, y: [C=128, N=4096], mu: [128,1]):
    xc = x - mu ; yc = y - mu                       (bf16)
    v[m] = 1/max(||yc[:,m]||, 1e-12); u[n] likewise for xc
    ycv = yc * v[m]  (v broadcast to a row via PE-transpose + DMA reshape +
                      Pool partition_broadcast)
    Sv = xc_blk^T @ ycv                             (PE -> PSUM = G*v)
    drain Sv -> SBUF bf16 (split: DVE tensor_copy chunk0 / ACT copy chunk1)
    gv[n] = rowmax Sv  (DVE pairwise tensor_tensor-max tree @2x)
    sc = 1/(h(1+eps) - h*u*gv); se = sc*u; bv = 1/h - sc
    w = exp(se*Sv + bv)  (ACT, fused rowsum accumulator -> r)
    wn = w / r           (DVE tensor_scalar @4x)
    colrow_nb[m] = max_n wn  (Pool partition_all_reduce) -> DMA to
                   collector partition nb
    tail: par over the 32 collector rows -> mean -> -log
"""

import numpy as np

import concourse.bass as bass
import concourse.tile as tile
from concourse import bacc, mybir
from concourse.bass_utils import run_bass_kernel_spmd

F32 = mybir.dt.float32
BF16 = mybir.dt.bfloat16
AF = mybir.ActivationFunctionType
OP = mybir.AluOpType
RED = bass.bass_isa.ReduceOp

B, C, H, W = 8, 128, 64, 64
N = H * W                  # 4096
P = 128
NBLK = N // P              # 32 row blocks
MM_N = 512                 # matmul moving free dim (one PSUM bank)
MC = 2048                  # PSUM chunk (4 banks); 2 chunks per block
DD = 1024                  # DVE drain width within chunk0 (rest + chunk1: ACT)
H_BW = 0.5
EPS = 1e-5
NORM_EPS = 1e-12
NEG_INF = -3.0e38
N_CORES = 8
PRE_CH = 1024              # preamble processing chunk


def _kernel_body(tc):
    nc = tc.nc
    x_d = nc.dram_tensor("x", [P, N], F32, kind="ExternalInput").ap()
    y_d = nc.dram_tensor("y", [P, N], F32, kind="ExternalInput").ap()
    mu_d = nc.dram_tensor("mu", [P, 1], F32, kind="ExternalInput").ap()
    id_d = nc.dram_tensor("ident", [P, P], F32, kind="ExternalInput").ap()
    coll_d = nc.dram_tensor("coll", [NBLK, N], BF16, kind="ExternalOutput").ap()
    vs_d = nc.dram_tensor("vscratch", [1, N], BF16, kind="Internal").ap()

    from contextlib import ExitStack
    with ExitStack() as ctx:
        persist = ctx.enter_context(tc.tile_pool(name="persist", bufs=1))
        small = ctx.enter_context(tc.tile_pool(name="small", bufs=8))

        # ---- constants ----
        mu_sb = persist.tile([P, 1], F32)
        nc.sync.dma_start(mu_sb[:], mu_d)
        negmu = persist.tile([P, 1], F32)
        nc.vector.tensor_scalar_mul(negmu[:], mu_sb[:], -1.0)
        ident_f = persist.tile([P, P], F32)
        nc.sync.dma_start(ident_f[:], id_d)
        ones_col_bf = persist.tile([P, 1], BF16)
        nc.vector.memset(ones_col_bf[:], 1.0)
        c_hbias = persist.tile([P, 1], F32)
        nc.vector.memset(c_hbias[:], H_BW * (1.0 + EPS))
        c_invh = persist.tile([P, 1], F32)
        nc.vector.memset(c_invh[:], 1.0 / H_BW)

        # preload ACT function tables (Identity/Sqrt/Exp) during the DMA wait
        warm = persist.tile([P, 1], F32)
        nc.scalar.activation(warm[:], c_hbias[:], AF.Identity,
                             bias=0.0, scale=1.0)
        nc.scalar.activation(warm[:], c_hbias[:], AF.Sqrt)
        nc.scalar.activation(warm[:], c_hbias[:], AF.Exp,
                             bias=0.0, scale=1.0)

        xc = persist.tile([P, N], BF16)
        ycv = persist.tile([P, N], BF16)
        u_col = persist.tile([P, NBLK], F32)
        hu_col = persist.tile([P, NBLK], F32)

        # ---------- preamble: load + center + norms ----------
        NCH = N // PRE_CH
        with tc.tile_pool(name="load", bufs=1) as load, \
             tc.tile_pool(name="ssq_ps", bufs=1, space="PSUM") as sp, \
             tc.tile_pool(name="sq_pool", bufs=2) as sqp:
            y_sb = load.tile([P, N], F32)
            x_sb = load.tile([P, N], F32)
            yc = load.tile([P, N], BF16)
            ssq_y = sp.tile([P, NBLK], F32)
            ssq_x = sp.tile([P, NBLK], F32)
            # y chain first (feeds v -> vrow -> ycv, the loop's rhs)
            for c in range(NCH):
                s = c * PRE_CH
                nc.sync.dma_start(y_sb[:, s:s + PRE_CH], y_d[:, s:s + PRE_CH])

            vseed = persist.tile([1, N], BF16)
            vrow = load.tile([P, N], BF16)
            v_col = load.tile([P, NBLK], F32)
            ident_bf = small.tile([P, P], BF16, name="ident_bf")
            nc.vector.tensor_copy(ident_bf[:], ident_f[:])
            JCH = PRE_CH // P            # ssq cols per chunk (8)
            for c in range(NCH):
                s = c * PRE_CH
                j0 = c * JCH
                # center on ACT (bias = -mu), square on DVE (bf16 2x)
                nc.scalar.activation(yc[:, s:s + PRE_CH], y_sb[:, s:s + PRE_CH],
                                     AF.Identity, bias=negmu[:, 0:1], scale=1.0)
                ysq = sqp.tile([P, PRE_CH], BF16, name="ysq")
                nc.vector.tensor_mul(ysq[:], yc[:, s:s + PRE_CH],
                                     yc[:, s:s + PRE_CH])
                for j in range(JCH):
                    col = j0 + j
                    nc.tensor.matmul(ssq_y[:, col:col + 1],
                                     ysq[:, j * P:(j + 1) * P],
                                     ones_col_bf[:], start=True, stop=True)
                # per-chunk v chain -> row seed -> broadcast -> ycv
                nc.scalar.activation(v_col[:, j0:j0 + JCH],
                                     ssq_y[:, j0:j0 + JCH], AF.Sqrt)
                nc.vector.tensor_scalar_max(v_col[:, j0:j0 + JCH],
                                            v_col[:, j0:j0 + JCH], NORM_EPS)
                nc.vector.reciprocal(v_col[:, j0:j0 + JCH],
                                     v_col[:, j0:j0 + JCH])
                with tc.tile_pool(name=f"vt_ps{c}", bufs=1, space="PSUM") as vtp:
                    vT_ps = vtp.tile([JCH, P], F32, name="vT_ps")
                    nc.tensor.transpose(vT_ps[:], v_col[:, j0:j0 + JCH],
                                        ident_f[:])
                    vT = small.tile([JCH, P], BF16, name="vT")
                    nc.vector.tensor_copy(vT[:], vT_ps[:])
                nc.sync.dma_start(
                    vs_d[:, s:s + PRE_CH].rearrange("o (j p) -> (o j) p", p=P),
                    vT[:])
                nc.sync.dma_start(vseed[0:1, s:s + PRE_CH],
                                  vs_d[:, s:s + PRE_CH])
                nc.gpsimd.partition_broadcast(vrow[:, s:s + PRE_CH],
                                              vseed[0:1, s:s + PRE_CH])
                nc.vector.tensor_mul(ycv[:, s:s + PRE_CH],
                                     yc[:, s:s + PRE_CH],
                                     vrow[:, s:s + PRE_CH])

            for c in range(NCH):
                s = c * PRE_CH
                nc.sync.dma_start(x_sb[:, s:s + PRE_CH], x_d[:, s:s + PRE_CH])
            # x chain (overlaps): center on DVE, square on DVE
            for c in range(NCH):
                s = c * PRE_CH
                j0 = c * JCH
                nc.vector.tensor_scalar_sub(xc[:, s:s + PRE_CH],
                                            x_sb[:, s:s + PRE_CH],
                                            mu_sb[:, 0:1])
                xsq = sqp.tile([P, PRE_CH], BF16, name="xsq")
                nc.vector.tensor_mul(xsq[:], xc[:, s:s + PRE_CH],
                                     xc[:, s:s + PRE_CH])
                for j in range(JCH):
                    col = j0 + j
                    nc.tensor.matmul(ssq_x[:, col:col + 1],
                                     xsq[:, j * P:(j + 1) * P],
                                     ones_col_bf[:], start=True, stop=True)
                nc.scalar.activation(u_col[:, j0:j0 + JCH],
                                     ssq_x[:, j0:j0 + JCH], AF.Sqrt)
                nc.vector.tensor_scalar_max(u_col[:, j0:j0 + JCH],
                                            u_col[:, j0:j0 + JCH], NORM_EPS)
                nc.vector.reciprocal(u_col[:, j0:j0 + JCH],
                                     u_col[:, j0:j0 + JCH])
                nc.vector.tensor_scalar_mul(hu_col[:, j0:j0 + JCH],
                                            u_col[:, j0:j0 + JCH], -H_BW)


        # ---------- main loop over 32 row blocks ----------
        with tc.tile_pool(name="g_ps_pool", bufs=2, space="PSUM") as gp, \
             tc.tile_pool(name="sv_pool", bufs=5) as svp, \
             tc.tile_pool(name="w_pool", bufs=5) as wp, \
             tc.tile_pool(name="tree_pool", bufs=3) as trp, \
             tc.tile_pool(name="par_pool", bufs=3) as pp:

            def block_exp(st):
                # exp(nb') for a previous block (ACT; overlaps this block's mm)
                nb_, sv_, se_, bv_ = st
                w = wp.tile([P, N], BF16, name="w")
                r = small.tile([P, 1], F32, name="r")
                nc.scalar.activation(w[:], sv_[:], AF.Exp, bias=bv_[:, 0:1],
                                     scale=se_[:, 0:1], accum_out=r[:, 0:1])
                return nb_, w, r

            def block_norm(st):
                # rinv/wn/par/dma for a previous block (DVE late + Pool)
                nb_, w, r = st
                rinv = small.tile([P, 1], F32, name="rinv")
                nc.vector.reciprocal(rinv[:], r[:])
                # wn = w * rinv in place (@4x)
                nc.vector.tensor_scalar_mul(w[:], w[:], rinv[:, 0:1])
                # column max over the block's 128 rows -> one row
                parout = pp.tile([P, N], BF16, name="parout")
                nc.gpsimd.partition_all_reduce(parout[:], w[:], P, RED.max)
                nc.sync.dma_start(coll_d[nb_:nb_ + 1, :], parout[0:1, :])

            prev = None       # block awaiting exp (1 behind)
            prev2 = None      # block awaiting rinv/wn/par (2 behind)
            for nb in range(NBLK):
                lhsT = xc[:, nb * P:(nb + 1) * P]
                sv = svp.tile([P, N], BF16, name="sv")
                for ch in range(N // MC):
                    g = gp.tile([P, MC], F32, name="g")
                    for j in range(MC // MM_N):
                        m0 = ch * MC + j * MM_N
                        nc.tensor.matmul(g[:, j * MM_N:(j + 1) * MM_N], lhsT,
                                         ycv[:, m0:m0 + MM_N],
                                         start=True, stop=True)
                    if ch == 0:
                        # drain split: DVE takes [0:DD]; ACT the rest
                        nc.vector.tensor_copy(sv[:, 0:DD], g[:, 0:DD])
                        nc.scalar.activation(sv[:, DD:MC], g[:, DD:MC],
                                             AF.Copy)
                    else:
                        nc.scalar.activation(sv[:, MC:N], g[:], AF.Copy)
                # exp(nb-1) AFTER the drains: keeps ACT's drains at higher
                # scheduler priority so the next block's tree starts early
                if prev is not None:
                    prev = block_exp(prev)

                # rowmax via pairwise tt-max tree (bf16 @2x)
                m1 = trp.tile([P, MC], BF16, name="m1")
                nc.vector.tensor_max(m1[:], sv[:, 0:MC], sv[:, MC:N])
                wdt = MC // 2
                while wdt >= 64:
                    nc.vector.tensor_max(m1[:, 0:wdt], m1[:, 0:wdt],
                                         m1[:, wdt:2 * wdt])
                    wdt //= 2
                gv = small.tile([P, 1], F32, name="gv")
                nc.vector.reduce_max(gv[:], m1[:, 0:64],
                                     axis=mybir.AxisListType.X)

                # small chain: sc = 1/(h(1+eps) - h*u*gv); se = sc*u;
                # bv = 1/h - sc
                t = small.tile([P, 1], F32, name="t")
                nc.vector.tensor_scalar(t[:], gv[:], hu_col[:, nb:nb + 1],
                                        c_hbias[:, 0:1], OP.mult, OP.add)
                sc = small.tile([P, 1], F32, name="sc")
                nc.vector.reciprocal(sc[:], t[:])
                se = small.tile([P, 1], F32, name="se")
                nc.vector.tensor_scalar_mul(se[:], sc[:], u_col[:, nb:nb + 1])
                bv = small.tile([P, 1], F32, name="bv")
                nc.vector.tensor_scalar(bv[:], sc[:], -1.0, c_invh[:, 0:1],
                                        OP.mult, OP.add)
                if prev2 is not None:
                    block_norm(prev2)  # DVE late: exp(nb-2) done long ago
                prev2 = prev
                prev = (nb, sv, se, bv)
            block_norm(prev2)          # block NBLK-2 (already exp'd)
            prev = block_exp(prev)     # block NBLK-1
            block_norm(prev)




_NC_CACHE = None


def _get_nc():
    global _NC_CACHE
    if _NC_CACHE is None:
        nc = bacc.Bacc("TRN2", target_bir_lowering=False, debug=False)
        with tile.TileContext(nc) as tc:
            _kernel_body(tc)
        nc.compile()
        _NC_CACHE = nc
    return _NC_CACHE


def kernel(inputs, targets):
    x = np.ascontiguousarray(np.asarray(inputs, dtype=np.float32))
    y = np.ascontiguousarray(np.asarray(targets, dtype=np.float32))
    assert x.shape == (B, C, H, W) and y.shape == (B, C, H, W)
    mu = y.mean(axis=(0, 2, 3)).astype(np.float32).reshape(C, 1)
    in_maps = [
        {
            "x": x[b].reshape(C, N),
            "y": y[b].reshape(C, N),
            "mu": mu,
            "ident": np.eye(P, dtype=np.float32),
        }
        for b in range(B)
    ]
    nc = _get_nc()
    res = run_bass_kernel_spmd(nc, in_maps, list(range(N_CORES)))
    losses = []
    for b in range(B):
        coll = np.asarray(res.results[b]["coll"], dtype=np.float32)
        cx = coll.max(axis=0).mean()
        losses.append(-np.log(cx + EPS))
    return np.float32(np.mean(losses))


# revision 5
# speedup vs baseline: 1.1332x; 1.0574x over previous
"""Trainium2 Bass kernel for ContextualLoss — v2.

Contract: kernel(**inputs) takes FULL inputs {"inputs": [8,128,64,64] f32,
"targets": [8,128,64,64] f32} and returns the FULL scalar loss (np.float32).

Sharding: data-parallel over batch B=8 across the 8 NeuronCores. Host
computes the cross-batch target channel mean y_mu and averages the 8
per-batch scalar losses.

Per-core math (x, y: [C=128, N=4096], mu: [128,1]):
    xc = x - mu ; yc = y - mu                       (bf16)
    v[m] = 1/max(||yc[:,m]||, 1e-12); u[n] likewise for xc
    ycv = yc * v[m]  (v broadcast to a row via PE-transpose + DMA reshape +
                      Pool partition_broadcast)
    Sv = xc_blk^T @ ycv                             (PE -> PSUM = G*v)
    drain Sv -> SBUF bf16 (split: DVE tensor_copy chunk0 / ACT copy chunk1)
    gv[n] = rowmax Sv  (DVE pairwise tensor_tensor-max tree @2x)
    sc = 1/(h(1+eps) - h*u*gv); se = sc*u; bv = 1/h - sc
    w = exp(se*Sv + bv)  (ACT, fused rowsum accumulator -> r)
    wn = w / r           (DVE tensor_scalar @4x)
    colrow_nb[m] = max_n wn  (Pool partition_all_reduce) -> DMA to
                   collector partition nb
    tail: par over the 32 collector rows -> mean -> -log
"""

import numpy as np

import concourse.bass as bass
import concourse.tile as tile
from concourse import bacc, mybir
from concourse.bass_utils import run_bass_kernel_spmd

F32 = mybir.dt.float32
BF16 = mybir.dt.bfloat16
AF = mybir.ActivationFunctionType
OP = mybir.AluOpType
RED = bass.bass_isa.ReduceOp

B, C, H, W = 8, 128, 64, 64
N = H * W                  # 4096
P = 128
NBLK = N // P              # 32 row blocks
MM_N = 512                 # matmul moving free dim (one PSUM bank)
MC = 2048                  # PSUM chunk (4 banks); 2 chunks per block
DD = 1024                  # DVE drain width within chunk0 (rest + chunk1: ACT)
H_BW = 0.5
EPS = 1e-5
NORM_EPS = 1e-12
NEG_INF = -3.0e38
N_CORES = 8
PRE_CH = 1024              # preamble processing chunk


def _kernel_body(tc):
    nc = tc.nc
    x_d = nc.dram_tensor("x", [P, N], F32, kind="ExternalInput").ap()
    y_d = nc.dram_tensor("y", [P, N], F32, kind="ExternalInput").ap()
    mu_d = nc.dram_tensor("mu", [P, 1], F32, kind="ExternalInput").ap()
    id_d = nc.dram_tensor("ident", [P, P], F32, kind="ExternalInput").ap()
    coll_d = nc.dram_tensor("coll", [NBLK, N], BF16, kind="ExternalOutput").ap()
    vs_d = nc.dram_tensor("vscratch", [1, N], BF16, kind="Internal").ap()

    from contextlib import ExitStack
    with ExitStack() as ctx:
        persist = ctx.enter_context(tc.tile_pool(name="persist", bufs=1))
        small = ctx.enter_context(tc.tile_pool(name="small", bufs=8))

        # ---- constants ----
        mu_sb = persist.tile([P, 1], F32)
        nc.sync.dma_start(mu_sb[:], mu_d)
        negmu = persist.tile([P, 1], F32)
        nc.vector.tensor_scalar_mul(negmu[:], mu_sb[:], -1.0)
        ident_f = persist.tile([P, P], F32)
        nc.sync.dma_start(ident_f[:], id_d)
        ones_col_bf = persist.tile([P, 1], BF16)
        nc.vector.memset(ones_col_bf[:], 1.0)
        c_hbias = persist.tile([P, 1], F32)
        nc.vector.memset(c_hbias[:], H_BW * (1.0 + EPS))
        c_invh = persist.tile([P, 1], F32)
        nc.vector.memset(c_invh[:], 1.0 / H_BW)

        # preload ACT function tables (Identity/Sqrt/Exp) during the DMA wait
        warm = persist.tile([P, 1], F32)
        nc.scalar.activation(warm[:], c_hbias[:], AF.Identity,
                             bias=0.0, scale=1.0)
        nc.scalar.activation(warm[:], c_hbias[:], AF.Sqrt)
        nc.scalar.activation(warm[:], c_hbias[:], AF.Exp,
                             bias=0.0, scale=1.0)

        xc = persist.tile([P, N], BF16)
        ycv = persist.tile([P, N], BF16)
        u_col = persist.tile([P, NBLK], F32)
        hu_col = persist.tile([P, NBLK], F32)

        # ---------- preamble: load + center + norms ----------
        NCH = N // PRE_CH
        with tc.tile_pool(name="load", bufs=1) as load, \
             tc.tile_pool(name="ssq_ps", bufs=1, space="PSUM") as sp, \
             tc.tile_pool(name="sq_pool", bufs=2) as sqp:
            y_sb = load.tile([P, N], F32)
            x_sb = load.tile([P, N], F32)
            yc = load.tile([P, N], BF16)
            ssq_y = sp.tile([P, NBLK], F32)
            ssq_x = sp.tile([P, NBLK], F32)
            # y chain first (feeds v -> vrow -> ycv, the loop's rhs)
            for c in range(NCH):
                s = c * PRE_CH
                nc.sync.dma_start(y_sb[:, s:s + PRE_CH], y_d[:, s:s + PRE_CH])

            vseed = persist.tile([1, N], BF16)
            vrow = load.tile([P, N], BF16)
            v_col = load.tile([P, NBLK], F32)
            ident_bf = small.tile([P, P], BF16, name="ident_bf")
            nc.vector.tensor_copy(ident_bf[:], ident_f[:])
            JCH = PRE_CH // P            # ssq cols per chunk (8)
            for c in range(NCH):
                s = c * PRE_CH
                j0 = c * JCH
                # center on ACT (bias = -mu), square on DVE (bf16 2x)
                nc.scalar.activation(yc[:, s:s + PRE_CH], y_sb[:, s:s + PRE_CH],
                                     AF.Identity, bias=negmu[:, 0:1], scale=1.0)
                ysq = sqp.tile([P, PRE_CH], BF16, name="ysq")
                nc.vector.tensor_mul(ysq[:], yc[:, s:s + PRE_CH],
                                     yc[:, s:s + PRE_CH])
                for j in range(JCH):
                    col = j0 + j
                    nc.tensor.matmul(ssq_y[:, col:col + 1],
                                     ysq[:, j * P:(j + 1) * P],
                                     ones_col_bf[:], start=True, stop=True)
                # per-chunk v chain -> row seed -> broadcast -> ycv
                nc.scalar.activation(v_col[:, j0:j0 + JCH],
                                     ssq_y[:, j0:j0 + JCH], AF.Sqrt)
                nc.vector.tensor_scalar_max(v_col[:, j0:j0 + JCH],
                                            v_col[:, j0:j0 + JCH], NORM_EPS)
                nc.vector.reciprocal(v_col[:, j0:j0 + JCH],
                                     v_col[:, j0:j0 + JCH])
                with tc.tile_pool(name=f"vt_ps{c}", bufs=1, space="PSUM") as vtp:
                    vT_ps = vtp.tile([JCH, P], F32, name="vT_ps")
                    nc.tensor.transpose(vT_ps[:], v_col[:, j0:j0 + JCH],
                                        ident_f[:])
                    vT = small.tile([JCH, P], BF16, name="vT")
                    nc.vector.tensor_copy(vT[:], vT_ps[:])
                nc.sync.dma_start(
                    vs_d[:, s:s + PRE_CH].rearrange("o (j p) -> (o j) p", p=P),
                    vT[:])
                nc.sync.dma_start(vseed[0:1, s:s + PRE_CH],
                                  vs_d[:, s:s + PRE_CH])
                nc.gpsimd.partition_broadcast(vrow[:, s:s + PRE_CH],
                                              vseed[0:1, s:s + PRE_CH])
                nc.vector.tensor_mul(ycv[:, s:s + PRE_CH],
                                     yc[:, s:s + PRE_CH],
                                     vrow[:, s:s + PRE_CH])

            for c in range(NCH):
                s = c * PRE_CH
                nc.sync.dma_start(x_sb[:, s:s + PRE_CH], x_d[:, s:s + PRE_CH])
            # x chain (overlaps): center on DVE, square on DVE
            for c in range(NCH):
                s = c * PRE_CH
                j0 = c * JCH
                nc.vector.tensor_scalar_sub(xc[:, s:s + PRE_CH],
                                            x_sb[:, s:s + PRE_CH],
                                            mu_sb[:, 0:1])
                xsq = sqp.tile([P, PRE_CH], BF16, name="xsq")
                nc.vector.tensor_mul(xsq[:], xc[:, s:s + PRE_CH],
                                     xc[:, s:s + PRE_CH])
                for j in range(JCH):
                    col = j0 + j
                    nc.tensor.matmul(ssq_x[:, col:col + 1],
                                     xsq[:, j * P:(j + 1) * P],
                                     ones_col_bf[:], start=True, stop=True)
                nc.scalar.activation(u_col[:, j0:j0 + JCH],
                                     ssq_x[:, j0:j0 + JCH], AF.Sqrt)
                nc.vector.tensor_scalar_max(u_col[:, j0:j0 + JCH],
                                            u_col[:, j0:j0 + JCH], NORM_EPS)
                nc.vector.reciprocal(u_col[:, j0:j0 + JCH],
                                     u_col[:, j0:j0 + JCH])
                nc.vector.tensor_scalar_mul(hu_col[:, j0:j0 + JCH],
                                            u_col[:, j0:j0 + JCH], -H_BW)


        # ---------- main loop over 32 row blocks ----------
        with tc.tile_pool(name="g_ps_pool", bufs=2, space="PSUM") as gp, \
             tc.tile_pool(name="sv_pool", bufs=5) as svp, \
             tc.tile_pool(name="w_pool", bufs=5) as wp, \
             tc.tile_pool(name="tree_pool", bufs=3) as trp, \
             tc.tile_pool(name="par_pool", bufs=3) as pp:

            def block_exp(st):
                # exp(nb') for a previous block (ACT; overlaps this block's mm)
                nb_, sv_, se_, bv_ = st
                w = wp.tile([P, N], BF16, name="w")
                r = small.tile([P, 1], F32, name="r")
                nc.scalar.activation(w[:], sv_[:], AF.Exp, bias=bv_[:, 0:1],
                                     scale=se_[:, 0:1], accum_out=r[:, 0:1])
                return nb_, w, r

            def block_norm(st):
                # rinv/wn/par/dma for a previous block (DVE late + Pool)
                nb_, w, r = st
                rinv = small.tile([P, 1], F32, name="rinv")
                nc.vector.reciprocal(rinv[:], r[:])
                # wn = w * rinv in place (@4x)
                nc.vector.tensor_scalar_mul(w[:], w[:], rinv[:, 0:1])
                # column max over the block's 128 rows -> one row
                parout = pp.tile([P, N], BF16, name="parout")
                nc.gpsimd.partition_all_reduce(parout[:], w[:], P, RED.max)
                nc.sync.dma_start(coll_d[nb_:nb_ + 1, :], parout[0:1, :])

            prev = None       # block awaiting exp (1 behind)
            prev2 = None      # block awaiting rinv/wn/par (2 behind)
            for nb in range(NBLK):
                lhsT = xc[:, nb * P:(nb + 1) * P]
                sv = svp.tile([P, N], BF16, name="sv")
                for ch in range(N // MC):
                    g = gp.tile([P, MC], F32, name="g")
                    for j in range(MC // MM_N):
                        m0 = ch * MC + j * MM_N
                        nc.tensor.matmul(g[:, j * MM_N:(j + 1) * MM_N], lhsT,
                                         ycv[:, m0:m0 + MM_N],
                                         start=True, stop=True)
                    if ch == 0:
                        # DVE drains all of chunk0; ACT only chunk1
                        nc.vector.tensor_copy(sv[:, 0:MC], g[:])
                    else:
                        nc.scalar.activation(sv[:, MC:N], g[:], AF.Copy)
                # exp(nb-1) AFTER the drains: keeps ACT's drains at higher
                # scheduler priority so the next block's tree starts early
                if prev is not None:
                    prev = block_exp(prev)

                # rowmax via pairwise tt-max tree (bf16 @2x)
                m1 = trp.tile([P, MC], BF16, name="m1")
                nc.vector.tensor_max(m1[:], sv[:, 0:MC], sv[:, MC:N])
                wdt = MC // 2
                while wdt >= 64:
                    nc.vector.tensor_max(m1[:, 0:wdt], m1[:, 0:wdt],
                                         m1[:, wdt:2 * wdt])
                    wdt //= 2
                gv = small.tile([P, 1], F32, name="gv")
                nc.vector.reduce_max(gv[:], m1[:, 0:64],
                                     axis=mybir.AxisListType.X)

                # small chain: sc = 1/(h(1+eps) - h*u*gv); se = sc*u;
                # bv = 1/h - sc
                t = small.tile([P, 1], F32, name="t")
                nc.vector.tensor_scalar(t[:], gv[:], hu_col[:, nb:nb + 1],
                                        c_hbias[:, 0:1], OP.mult, OP.add)
                sc = small.tile([P, 1], F32, name="sc")
                nc.vector.reciprocal(sc[:], t[:])
                se = small.tile([P, 1], F32, name="se")
                nc.vector.tensor_scalar_mul(se[:], sc[:], u_col[:, nb:nb + 1])
                bv = small.tile([P, 1], F32, name="bv")
                nc.vector.tensor_scalar(bv[:], sc[:], -1.0, c_invh[:, 0:1],
                                        OP.mult, OP.add)
                if prev2 is not None:
                    block_norm(prev2)  # DVE late: exp(nb-2) done long ago
                prev2 = prev
                prev = (nb, sv, se, bv)
            block_norm(prev2)          # block NBLK-2 (already exp'd)
            prev = block_exp(prev)     # block NBLK-1
            block_norm(prev)




_NC_CACHE = None


def _get_nc():
    global _NC_CACHE
    if _NC_CACHE is None:
        nc = bacc.Bacc("TRN2", target_bir_lowering=False, debug=False)
        with tile.TileContext(nc) as tc:
            _kernel_body(tc)
        nc.compile()
        _NC_CACHE = nc
    return _NC_CACHE


def kernel(inputs, targets):
    x = np.ascontiguousarray(np.asarray(inputs, dtype=np.float32))
    y = np.ascontiguousarray(np.asarray(targets, dtype=np.float32))
    assert x.shape == (B, C, H, W) and y.shape == (B, C, H, W)
    mu = y.mean(axis=(0, 2, 3)).astype(np.float32).reshape(C, 1)
    in_maps = [
        {
            "x": x[b].reshape(C, N),
            "y": y[b].reshape(C, N),
            "mu": mu,
            "ident": np.eye(P, dtype=np.float32),
        }
        for b in range(B)
    ]
    nc = _get_nc()
    res = run_bass_kernel_spmd(nc, in_maps, list(range(N_CORES)))
    losses = []
    for b in range(B):
        coll = np.asarray(res.results[b]["coll"], dtype=np.float32)
        cx = coll.max(axis=0).mean()
        losses.append(-np.log(cx + EPS))
    return np.float32(np.mean(losses))
